# revision 9
# baseline (speedup 1.0000x reference)
"""NonLocalBlock (embedded-gaussian attention) TRN2 kernel.

Shapes (hardcoded): x [8, 256, 64, 64] fp32.
Per batch element b (one NeuronCore each, 8 cores data-parallel):
  theta/phi/g = 1x1 conv projections of x_b [256, 4096] -> [128, 4096]
  f^T[j, i] = sum_c phi[c, j] theta[c, i]        (4096 x 4096 logits)
  soft = softmax over j  (no max subtraction: |f| <= ~91, exp fits fp32)
  y[ci, i] = sum_j soft[j, i] gT[j, ci]          (normalization deferred)
  out = x + W_w @ (y / Z) + (W_w @ g_b + W_b)    (g bias folded via softmax sum=1)

Device layout notes:
  - fT computed j-block (128) x i-quarter (1024) at a time; exp on ScalarE
    (PSUM -> SBUF); y accumulated in PSUM over all 32 j-blocks.
  - Softmax denominator Z: DVE accumulates expf over j-blocks (Zacc), PE
    ones-matmul reduces the 128 partitions per quarter; some j-blocks can be
    reduced directly on PE (PE_Z_JS) to balance engines.
  - Reciprocal of Z broadcast to 128 partitions via stride-0 DMA.
  - All matmuls in float32r (1 col/cycle; ~tf32 precision).
"""

import numpy as np

import concourse.bacc as bacc
import concourse.mybir as mybir
from concourse import tile
from concourse.bass_utils import run_bass_kernel_spmd

F32 = mybir.dt.float32
F32R = mybir.dt.float32r
AF = mybir.ActivationFunctionType
ALU = mybir.AluOpType

B, C, CI = 8, 256, 128
H, Wd = 64, 64
N = H * Wd              # 4096
NQ = 4                  # i-quarters
QW = N // NQ            # 1024
JB = N // 128           # 32 j-blocks

# j-blocks whose Z-reduction runs on PE (ones-matmul) instead of DVE adds.
PE_Z_JS = frozenset()


def build(pe_z_js=PE_Z_JS):
    nc = bacc.Bacc("TRN2", target_bir_lowering=False, debug=False, num_devices=8)

    x_d = nc.dram_tensor("x", [C, N], F32R, kind="ExternalInput")
    thw_d = nc.dram_tensor("thw_t", [C, CI], F32R, kind="ExternalInput")  # theta_w.T
    phw_d = nc.dram_tensor("phw_t", [C, CI], F32R, kind="ExternalInput")  # phi_w.T
    gw_d = nc.dram_tensor("gw_t", [C, CI], F32R, kind="ExternalInput")    # g_w.T
    ww_d = nc.dram_tensor("ww_t", [CI, C], F32R, kind="ExternalInput")    # W_w.T
    thb_d = nc.dram_tensor("thb", [CI, 1], F32, kind="ExternalInput")
    phb_d = nc.dram_tensor("phb", [CI, 1], F32, kind="ExternalInput")
    wbe_d = nc.dram_tensor("wb_eff", [C, 1], F32, kind="ExternalInput")   # W_w@g_b + W_b
    ones_d = nc.dram_tensor("ones", [128, 1], F32R, kind="ExternalInput")
    out_d = nc.dram_tensor("out", [C, N], F32, kind="ExternalOutput")

    with tile.TileContext(nc) as tc:
        with (
            tc.tile_pool(name="const", bufs=1) as cpool,
            tc.tile_pool(name="big", bufs=1) as bigpool,
            tc.tile_pool(name="ef", bufs=4) as efpool,
            tc.tile_pool(name="zpool", bufs=2) as zpool,
            tc.tile_pool(name="ypool", bufs=2) as ypool,
            tc.tile_pool(name="opool", bufs=4) as opool,
            tc.tile_pool(name="pf", bufs=2, space="PSUM") as pf,
            tc.tile_pool(name="py", bufs=1, space="PSUM") as py,
            tc.tile_pool(name="pz", bufs=2, space="PSUM") as pz,
        ):
            # ---------------- weight / input loads ----------------
            thw = cpool.tile([128, 2 * CI], F32R, tag="thw")
            phw = cpool.tile([128, 2 * CI], F32R, tag="phw")
            gw = cpool.tile([128, 2 * CI], F32R, tag="gw")
            for t, d in ((thw, thw_d), (phw, phw_d), (gw, gw_d)):
                nc.sync.dma_start(t[:, 0:CI], d[0:128, :])
                nc.sync.dma_start(t[:, CI:2 * CI], d[128:256, :])
            ww = cpool.tile([CI, C], F32R, tag="ww")
            nc.sync.dma_start(ww[:], ww_d[:])
            thb = cpool.tile([CI, 1], F32, tag="thb")
            nc.sync.dma_start(thb[:], thb_d[:])
            phb = cpool.tile([CI, 1], F32, tag="phb")
            nc.sync.dma_start(phb[:], phb_d[:])
            wbe0 = cpool.tile([128, 1], F32, tag="wbe0")
            nc.sync.dma_start(wbe0[:], wbe_d[0:128, :])
            wbe1 = cpool.tile([128, 1], F32, tag="wbe1")
            nc.sync.dma_start(wbe1[:], wbe_d[128:256, :])
            ones_col = cpool.tile([128, 1], F32R, tag="ones")
            nc.sync.dma_start(ones_col[:], ones_d[:])

            x0 = bigpool.tile([128, N], F32R, tag="x0")
            nc.sync.dma_start(x0[:], x_d[0:128, :])
            x1 = bigpool.tile([128, N], F32R, tag="x1")
            nc.sync.dma_start(x1[:], x_d[128:256, :])
            xs = (x0, x1)

            th_sb = bigpool.tile([128, N], F32R, tag="th")
            ph_sb = bigpool.tile([128, N], F32R, tag="ph")
            gT_sb = bigpool.tile([128, N], F32R, tag="gT")

            # ---------------- projections: theta, phi ----------------
            for wt, bias_t, dst in ((thw, thb, th_sb), (phw, phb, ph_sb)):
                for p in range(N // QW):
                    pp = pf.tile([128, QW], F32, tag="pf")
                    for s in range(2):
                        lo = p * QW + s * 512
                        for k in range(2):
                            nc.tensor.matmul(
                                pp[:, s * 512:(s + 1) * 512],
                                wt[:, k * CI:(k + 1) * CI],
                                xs[k][:, lo:lo + 512],
                                start=(k == 0), stop=(k == 1),
                            )
                    nc.scalar.activation(
                        dst[:, p * QW:(p + 1) * QW], pp[:], AF.Identity, bias=bias_t[:]
                    )

            # ---------------- projection: gT (bias folded into wb_eff) ----------
            for j in range(JB):
                pg = pf.tile([128, 128], F32, tag="pf")
                for k in range(2):
                    nc.tensor.matmul(
                        pg[:],
                        xs[k][:, j * 128:(j + 1) * 128],
                        gw[:, k * CI:(k + 1) * CI],
                        start=(k == 0), stop=(k == 1),
                    )
                nc.scalar.activation(
                    gT_sb[:, j * 128:(j + 1) * 128], pg[:], AF.Copy
                )

            # x := x + (W_w @ g_b + W_b), per-partition scalar (after all
            # projection reads of x).
            nc.vector.tensor_scalar_add(x0[:], x0[:], wbe0[:])
            nc.vector.tensor_scalar_add(x1[:], x1[:], wbe1[:])

            # ---------------- main attention loop ----------------
            for q in range(NQ):
                i0 = q * QW
                pyt = py.tile([128, QW], F32, tag="py")
                zacc = zpool.tile([128, QW], F32R, tag="zacc")
                pzt = [None, None]
                pe_z_done = [False, False]
                n_dve = 0
                for j in range(JB):
                    pft = pf.tile([128, QW], F32, tag="pf")
                    for s in range(2):
                        nc.tensor.matmul(
                            pft[:, s * 512:(s + 1) * 512],
                            ph_sb[:, j * 128:(j + 1) * 128],
                            th_sb[:, i0 + s * 512:i0 + (s + 1) * 512],
                            start=True, stop=True,
                        )
                    ef = efpool.tile([128, QW], F32R, tag="ef")
                    nc.scalar.activation(ef[:], pft[:], AF.Exp)
                    for s in range(2):
                        nc.tensor.matmul(
                            pyt[:, s * 512:(s + 1) * 512],
                            gT_sb[:, j * 128:(j + 1) * 128],
                            ef[:, s * 512:(s + 1) * 512],
                            start=(j == 0), stop=(j == JB - 1),
                        )
                    if j in pe_z_js:
                        for s in range(2):
                            if pzt[s] is None:
                                pzt[s] = pz.tile([1, 512], F32, tag="pz",
                                                 name=f"pz_{q}_{s}")
                            nc.tensor.matmul(
                                pzt[s][:], ones_col[:],
                                ef[:, s * 512:(s + 1) * 512],
                                start=(not pe_z_done[s]), stop=False,
                            )
                            pe_z_done[s] = True
                    else:
                        if n_dve == 0:
                            nc.vector.tensor_copy(zacc[:], ef[:])
                        else:
                            nc.vector.tensor_add(zacc[:], zacc[:], ef[:])
                        n_dve += 1

                # ---- quarter tail: finish Z, normalize, project, add x ----
                zi = zpool.tile([1, QW], F32, tag="zi")
                for s in range(2):
                    if pzt[s] is None:
                        pzt[s] = pz.tile([1, 512], F32, tag="pz",
                                         name=f"pz_{q}_{s}")
                    if n_dve:
                        nc.tensor.matmul(
                            pzt[s][:], ones_col[:],
                            zacc[:, s * 512:(s + 1) * 512],
                            start=(not pe_z_done[s]), stop=True,
                        )
                    nc.vector.reciprocal(zi[:, s * 512:(s + 1) * 512], pzt[s][:])
                zb = zpool.tile([128, QW], F32, tag="zb")
                nc.gpsimd.partition_broadcast(zb[:], zi[:])

                yt = ypool.tile([128, QW], F32R, tag="yt")
                nc.vector.tensor_copy(yt[:], pyt[:])  # frees py slot quickly
                ynt = ypool.tile([128, QW], F32R, tag="ynt")
                nc.vector.tensor_mul(ynt[:], yt[:], zb[:])

                for s2 in range(2):
                    pw = py.tile([128, QW], F32, tag="py")
                    for ob in range(2):
                        nc.tensor.matmul(
                            pw[:, ob * 512:(ob + 1) * 512],
                            ww[:, ob * CI:(ob + 1) * CI],
                            ynt[:, s2 * 512:(s2 + 1) * 512],
                            start=True, stop=True,
                        )
                    for ob in range(2):
                        ot = opool.tile([128, 512], F32, tag="o")
                        nc.vector.tensor_add(
                            ot[:], pw[:, ob * 512:(ob + 1) * 512],
                            xs[ob][:, i0 + s2 * 512:i0 + (s2 + 1) * 512],
                        )
                        nc.sync.dma_start(
                            out_d[ob * 128:(ob + 1) * 128,
                                  i0 + s2 * 512:i0 + (s2 + 1) * 512],
                            ot[:],
                        )

    nc.compile()
    return nc


_CACHE = {}


def _get_nc():
    if "nc" not in _CACHE:
        _CACHE["nc"] = build()
    return _CACHE["nc"]


def _in_maps(x, g_w, g_b, theta_w, theta_b, phi_w, phi_b, W_w, W_b):
    x = np.ascontiguousarray(np.asarray(x, dtype=np.float32))
    common = {
        "thw_t": np.ascontiguousarray(np.asarray(theta_w, np.float32).T),
        "phw_t": np.ascontiguousarray(np.asarray(phi_w, np.float32).T),
        "gw_t": np.ascontiguousarray(np.asarray(g_w, np.float32).T),
        "ww_t": np.ascontiguousarray(np.asarray(W_w, np.float32).T),
        "thb": np.asarray(theta_b, np.float32).reshape(CI, 1),
        "phb": np.asarray(phi_b, np.float32).reshape(CI, 1),
        "wb_eff": (np.asarray(W_w, np.float32) @ np.asarray(g_b, np.float32)
                   + np.asarray(W_b, np.float32)).reshape(C, 1),
        "ones": np.ones((128, 1), np.float32),
    }
    return [
        {"x": np.ascontiguousarray(x[b].reshape(C, N)), **common}
        for b in range(B)
    ]


def run(in_maps, **kw):
    nc = _get_nc()
    return run_bass_kernel_spmd(nc, in_maps, list(range(B)), **kw)


def kernel(**inputs):
    res = run(_in_maps(**inputs))
    out = np.stack([res.results[b]["out"] for b in range(B)])
    return out.reshape(B, C, H, Wd)


# revision 12
# speedup vs baseline: 1.0199x; 1.0199x over previous
"""NonLocalBlock (embedded-gaussian attention) TRN2 kernel.

Shapes (hardcoded): x [8, 256, 64, 64] fp32.
Per batch element b (one NeuronCore each, 8 cores data-parallel):
  theta/phi/g = 1x1 conv projections of x_b [256, 4096] -> [128, 4096]
  f^T[j, i] = sum_c phi[c, j] theta[c, i]        (4096 x 4096 logits)
  soft = softmax over j  (no max subtraction: |f| <= ~91, exp fits fp32)
  y[ci, i] = sum_j soft[j, i] gT[j, ci]          (normalization deferred)
  out = x + W_w @ (y / Z) + (W_w @ g_b + W_b)    (g bias folded via softmax sum=1)

Device layout notes:
  - fT computed j-block (128) x i-quarter (1024) at a time; exp on ScalarE
    (PSUM -> SBUF); y accumulated in PSUM over all 32 j-blocks.
  - Softmax denominator Z: DVE accumulates expf over j-blocks (Zacc), PE
    ones-matmul reduces the 128 partitions per quarter; some j-blocks can be
    reduced directly on PE (PE_Z_JS) to balance engines.
  - Reciprocal of Z broadcast to 128 partitions via stride-0 DMA.
  - All matmuls in float32r (1 col/cycle; ~tf32 precision).
"""

import numpy as np

import concourse.bacc as bacc
import concourse.mybir as mybir
from concourse import tile
from concourse.bass_utils import run_bass_kernel_spmd

F32 = mybir.dt.float32
F32R = mybir.dt.float32r
AF = mybir.ActivationFunctionType
ALU = mybir.AluOpType

B, C, CI = 8, 256, 128
H, Wd = 64, 64
N = H * Wd              # 4096
NQ = 4                  # i-quarters
QW = N // NQ            # 1024
JB = N // 128           # 32 j-blocks

# j-blocks whose Z-reduction runs on PE (ones-matmul) instead of DVE adds.
PE_Z_JS = frozenset(range(0, JB, 3))  # ~1/3 on PE


def build(pe_z_js=PE_Z_JS):
    nc = bacc.Bacc("TRN2", target_bir_lowering=False, debug=False, num_devices=8)

    x_d = nc.dram_tensor("x", [C, N], F32R, kind="ExternalInput")
    thw_d = nc.dram_tensor("thw_t", [C, CI], F32R, kind="ExternalInput")  # theta_w.T
    phw_d = nc.dram_tensor("phw_t", [C, CI], F32R, kind="ExternalInput")  # phi_w.T
    gw_d = nc.dram_tensor("gw_t", [C, CI], F32R, kind="ExternalInput")    # g_w.T
    ww_d = nc.dram_tensor("ww_t", [CI, C], F32R, kind="ExternalInput")    # W_w.T
    thb_d = nc.dram_tensor("thb", [CI, 1], F32, kind="ExternalInput")
    phb_d = nc.dram_tensor("phb", [CI, 1], F32, kind="ExternalInput")
    wbe_d = nc.dram_tensor("wb_eff", [C, 1], F32, kind="ExternalInput")   # W_w@g_b + W_b
    ones_d = nc.dram_tensor("ones", [128, 1], F32R, kind="ExternalInput")
    out_d = nc.dram_tensor("out", [C, N], F32, kind="ExternalOutput")

    with tile.TileContext(nc) as tc:
        with (
            tc.tile_pool(name="const", bufs=1) as cpool,
            tc.tile_pool(name="big", bufs=1) as bigpool,
            tc.tile_pool(name="ef", bufs=6) as efpool,
            tc.tile_pool(name="zpool", bufs=2) as zpool,
            tc.tile_pool(name="ypool", bufs=2) as ypool,
            tc.tile_pool(name="opool", bufs=4) as opool,
            tc.tile_pool(name="pf", bufs=2, space="PSUM") as pf,
            tc.tile_pool(name="py", bufs=1, space="PSUM") as py,
            tc.tile_pool(name="pz", bufs=2, space="PSUM") as pz,
        ):
            # ---------------- weight / input loads ----------------
            thw = cpool.tile([128, 2 * CI], F32R, tag="thw")
            phw = cpool.tile([128, 2 * CI], F32R, tag="phw")
            gw = cpool.tile([128, 2 * CI], F32R, tag="gw")
            for t, d in ((thw, thw_d), (phw, phw_d), (gw, gw_d)):
                nc.sync.dma_start(t[:, 0:CI], d[0:128, :])
                nc.sync.dma_start(t[:, CI:2 * CI], d[128:256, :])
            ww = cpool.tile([CI, C], F32R, tag="ww")
            nc.sync.dma_start(ww[:], ww_d[:])
            thb = cpool.tile([CI, 1], F32, tag="thb")
            nc.sync.dma_start(thb[:], thb_d[:])
            phb = cpool.tile([CI, 1], F32, tag="phb")
            nc.sync.dma_start(phb[:], phb_d[:])
            wbe0 = cpool.tile([128, 1], F32, tag="wbe0")
            nc.sync.dma_start(wbe0[:], wbe_d[0:128, :])
            wbe1 = cpool.tile([128, 1], F32, tag="wbe1")
            nc.sync.dma_start(wbe1[:], wbe_d[128:256, :])
            ones_col = cpool.tile([128, 1], F32R, tag="ones")
            nc.sync.dma_start(ones_col[:], ones_d[:])

            x0 = bigpool.tile([128, N], F32R, tag="x0")
            nc.sync.dma_start(x0[:], x_d[0:128, :])
            x1 = bigpool.tile([128, N], F32R, tag="x1")
            nc.sync.dma_start(x1[:], x_d[128:256, :])
            xs = (x0, x1)

            th_sb = bigpool.tile([128, N], F32R, tag="th")
            ph_sb = bigpool.tile([128, N], F32R, tag="ph")
            gT_sb = bigpool.tile([128, N], F32R, tag="gT")

            # ---------------- projections: theta, phi ----------------
            for wt, bias_t, dst in ((thw, thb, th_sb), (phw, phb, ph_sb)):
                for p in range(N // QW):
                    pp = pf.tile([128, QW], F32, tag="pf")
                    for s in range(2):
                        lo = p * QW + s * 512
                        for k in range(2):
                            nc.tensor.matmul(
                                pp[:, s * 512:(s + 1) * 512],
                                wt[:, k * CI:(k + 1) * CI],
                                xs[k][:, lo:lo + 512],
                                start=(k == 0), stop=(k == 1),
                            )
                    nc.scalar.activation(
                        dst[:, p * QW:(p + 1) * QW], pp[:], AF.Identity, bias=bias_t[:]
                    )

            # ---------------- projection: gT (bias folded into wb_eff) ----------
            for j in range(JB):
                pg = pf.tile([128, 128], F32, tag="pf")
                for k in range(2):
                    nc.tensor.matmul(
                        pg[:],
                        xs[k][:, j * 128:(j + 1) * 128],
                        gw[:, k * CI:(k + 1) * CI],
                        start=(k == 0), stop=(k == 1),
                    )
                nc.scalar.activation(
                    gT_sb[:, j * 128:(j + 1) * 128], pg[:], AF.Copy
                )

            # x := x + (W_w @ g_b + W_b), per-partition scalar (after all
            # projection reads of x).
            nc.vector.tensor_scalar_add(x0[:], x0[:], wbe0[:])
            nc.vector.tensor_scalar_add(x1[:], x1[:], wbe1[:])

            # ---------------- main attention loop ----------------
            for q in range(NQ):
                i0 = q * QW
                pyt = py.tile([128, QW], F32, tag="py")
                zacc = zpool.tile([128, QW], F32R, tag="zacc")
                pzt = [None, None]
                pe_z_done = [False, False]
                n_dve = 0
                for j in range(JB):
                    pft = pf.tile([128, QW], F32, tag="pf")
                    for s in range(2):
                        nc.tensor.matmul(
                            pft[:, s * 512:(s + 1) * 512],
                            ph_sb[:, j * 128:(j + 1) * 128],
                            th_sb[:, i0 + s * 512:i0 + (s + 1) * 512],
                            start=True, stop=True,
                        )
                    ef = efpool.tile([128, QW], F32R, tag="ef")
                    nc.scalar.activation(ef[:], pft[:], AF.Exp)
                    for s in range(2):
                        nc.tensor.matmul(
                            pyt[:, s * 512:(s + 1) * 512],
                            gT_sb[:, j * 128:(j + 1) * 128],
                            ef[:, s * 512:(s + 1) * 512],
                            start=(j == 0), stop=(j == JB - 1),
                        )
                    if j in pe_z_js:
                        for s in range(2):
                            if pzt[s] is None:
                                pzt[s] = pz.tile([1, 512], F32, tag="pz",
                                                 name=f"pz_{q}_{s}")
                            nc.tensor.matmul(
                                pzt[s][:], ones_col[:],
                                ef[:, s * 512:(s + 1) * 512],
                                start=(not pe_z_done[s]), stop=False,
                            )
                            pe_z_done[s] = True
                    else:
                        if n_dve == 0:
                            nc.vector.tensor_copy(zacc[:], ef[:])
                        else:
                            nc.vector.tensor_add(zacc[:], zacc[:], ef[:])
                        n_dve += 1

                # ---- quarter tail: finish Z, normalize, project, add x ----
                zi = zpool.tile([1, QW], F32, tag="zi")
                zs = zpool.tile([1, QW], F32, tag="zs")  # recip scratch
                for s in range(2):
                    if pzt[s] is None:
                        pzt[s] = pz.tile([1, 512], F32, tag="pz",
                                         name=f"pz_{q}_{s}")
                    if n_dve:
                        nc.tensor.matmul(
                            pzt[s][:], ones_col[:],
                            zacc[:, s * 512:(s + 1) * 512],
                            start=(not pe_z_done[s]), stop=True,
                        )
                    nc.vector.reciprocal_approx_accurate(
                        zi[:, s * 512:(s + 1) * 512], pzt[s][:],
                        zs[:, s * 512:(s + 1) * 512],
                    )
                zb = zpool.tile([128, QW], F32, tag="zb")
                nc.gpsimd.partition_broadcast(zb[:], zi[:])

                # fused eviction + normalization: ynt = psum_y * (1/Z)
                ynt = ypool.tile([128, QW], F32R, tag="ynt")
                nc.vector.tensor_mul(ynt[:], pyt[:], zb[:])

                for s2 in range(2):
                    pw = py.tile([128, QW], F32, tag="py")
                    for ob in range(2):
                        nc.tensor.matmul(
                            pw[:, ob * 512:(ob + 1) * 512],
                            ww[:, ob * CI:(ob + 1) * CI],
                            ynt[:, s2 * 512:(s2 + 1) * 512],
                            start=True, stop=True,
                        )
                    for ob in range(2):
                        ot = opool.tile([128, 512], F32, tag="o")
                        nc.vector.tensor_add(
                            ot[:], pw[:, ob * 512:(ob + 1) * 512],
                            xs[ob][:, i0 + s2 * 512:i0 + (s2 + 1) * 512],
                        )
                        nc.sync.dma_start(
                            out_d[ob * 128:(ob + 1) * 128,
                                  i0 + s2 * 512:i0 + (s2 + 1) * 512],
                            ot[:],
                        )

    nc.compile()
    return nc


_CACHE = {}


def _get_nc():
    if "nc" not in _CACHE:
        _CACHE["nc"] = build()
    return _CACHE["nc"]


def _in_maps(x, g_w, g_b, theta_w, theta_b, phi_w, phi_b, W_w, W_b):
    x = np.ascontiguousarray(np.asarray(x, dtype=np.float32))
    common = {
        "thw_t": np.ascontiguousarray(np.asarray(theta_w, np.float32).T),
        "phw_t": np.ascontiguousarray(np.asarray(phi_w, np.float32).T),
        "gw_t": np.ascontiguousarray(np.asarray(g_w, np.float32).T),
        "ww_t": np.ascontiguousarray(np.asarray(W_w, np.float32).T),
        "thb": np.asarray(theta_b, np.float32).reshape(CI, 1),
        "phb": np.asarray(phi_b, np.float32).reshape(CI, 1),
        "wb_eff": (np.asarray(W_w, np.float32) @ np.asarray(g_b, np.float32)
                   + np.asarray(W_b, np.float32)).reshape(C, 1),
        "ones": np.ones((128, 1), np.float32),
    }
    return [
        {"x": np.ascontiguousarray(x[b].reshape(C, N)), **common}
        for b in range(B)
    ]


def run(in_maps, **kw):
    nc = _get_nc()
    return run_bass_kernel_spmd(nc, in_maps, list(range(B)), **kw)


def kernel(**inputs):
    res = run(_in_maps(**inputs))
    out = np.stack([res.results[b]["out"] for b in range(B)])
    return out.reshape(B, C, H, Wd)


# revision 14
# speedup vs baseline: 1.0666x; 1.0458x over previous
"""NonLocalBlock (embedded-gaussian attention) TRN2 kernel.

Shapes (hardcoded): x [8, 256, 64, 64] fp32.
Per batch element b (one NeuronCore each, 8 cores data-parallel):
  theta/phi/g = 1x1 conv projections of x_b [256, 4096] -> [128, 4096]
  f^T[j, i] = sum_c phi[c, j] theta[c, i]        (4096 x 4096 logits)
  soft = softmax over j  (no max subtraction: |f| <= ~91, exp fits fp32)
  y[ci, i] = sum_j soft[j, i] gT[j, ci]          (normalization deferred)
  out = x + W_w @ (y / Z) + (W_w @ g_b + W_b)    (g bias folded via softmax sum=1)

Device layout notes:
  - fT computed j-block (128) x i-quarter (1024) at a time; exp on ScalarE
    (PSUM -> SBUF); y accumulated in PSUM over all 32 j-blocks.
  - Softmax denominator Z: DVE accumulates expf over j-blocks (Zacc), PE
    ones-matmul reduces the 128 partitions per quarter; some j-blocks can be
    reduced directly on PE (PE_Z_JS) to balance engines.
  - Reciprocal of Z broadcast to 128 partitions via stride-0 DMA.
  - All matmuls in float32r (1 col/cycle; ~tf32 precision).
"""

import numpy as np

import concourse.bacc as bacc
import concourse.mybir as mybir
from concourse import tile
from concourse.bass_utils import run_bass_kernel_spmd

F32 = mybir.dt.float32
F32R = mybir.dt.float32r
AF = mybir.ActivationFunctionType
ALU = mybir.AluOpType

B, C, CI = 8, 256, 128
H, Wd = 64, 64
N = H * Wd              # 4096
NQ = 4                  # i-quarters
QW = N // NQ            # 1024
JB = N // 128           # 32 j-blocks

# j-blocks whose Z-reduction runs on PE (ones-matmul) instead of DVE adds.
PE_Z_JS = frozenset(j for j in range(JB) if j % 5 < 2)  # ~40% on PE


def build(pe_z_js=PE_Z_JS):
    nc = bacc.Bacc("TRN2", target_bir_lowering=False, debug=False, num_devices=8)

    x_d = nc.dram_tensor("x", [C, N], F32R, kind="ExternalInput")
    thw_d = nc.dram_tensor("thw_t", [C, CI], F32R, kind="ExternalInput")  # theta_w.T
    phw_d = nc.dram_tensor("phw_t", [C, CI], F32R, kind="ExternalInput")  # phi_w.T
    gw_d = nc.dram_tensor("gw_t", [C, CI], F32R, kind="ExternalInput")    # g_w.T
    ww_d = nc.dram_tensor("ww_t", [CI, C], F32R, kind="ExternalInput")    # W_w.T
    thb_d = nc.dram_tensor("thb", [CI, 1], F32, kind="ExternalInput")
    phb_d = nc.dram_tensor("phb", [CI, 1], F32, kind="ExternalInput")
    wbe_d = nc.dram_tensor("wb_eff", [C, 1], F32, kind="ExternalInput")   # W_w@g_b + W_b
    ones_d = nc.dram_tensor("ones", [128, 1], F32R, kind="ExternalInput")
    out_d = nc.dram_tensor("out", [C, N], F32, kind="ExternalOutput")

    with tile.TileContext(nc) as tc:
        with (
            tc.tile_pool(name="const", bufs=1) as cpool,
            tc.tile_pool(name="big", bufs=1) as bigpool,
            tc.tile_pool(name="ef", bufs=10) as efpool,
            tc.tile_pool(name="zpool", bufs=2) as zpool,
            tc.tile_pool(name="ypool", bufs=2) as ypool,
            tc.tile_pool(name="opool", bufs=4) as opool,
            tc.tile_pool(name="pf", bufs=2, space="PSUM") as pf,
            tc.tile_pool(name="py", bufs=1, space="PSUM") as py,
            tc.tile_pool(name="pz", bufs=2, space="PSUM") as pz,
        ):
            # ---------------- weight / input loads ----------------
            thw = cpool.tile([128, 2 * CI], F32R, tag="thw")
            phw = cpool.tile([128, 2 * CI], F32R, tag="phw")
            gw = cpool.tile([128, 2 * CI], F32R, tag="gw")
            for t, d in ((thw, thw_d), (phw, phw_d), (gw, gw_d)):
                nc.sync.dma_start(t[:, 0:CI], d[0:128, :])
                nc.sync.dma_start(t[:, CI:2 * CI], d[128:256, :])
            ww = cpool.tile([CI, C], F32R, tag="ww")
            nc.sync.dma_start(ww[:], ww_d[:])
            thb = cpool.tile([CI, 1], F32, tag="thb")
            nc.sync.dma_start(thb[:], thb_d[:])
            phb = cpool.tile([CI, 1], F32, tag="phb")
            nc.sync.dma_start(phb[:], phb_d[:])
            wbe0 = cpool.tile([128, 1], F32, tag="wbe0")
            nc.sync.dma_start(wbe0[:], wbe_d[0:128, :])
            wbe1 = cpool.tile([128, 1], F32, tag="wbe1")
            nc.sync.dma_start(wbe1[:], wbe_d[128:256, :])
            ones_col = cpool.tile([128, 1], F32R, tag="ones")
            nc.sync.dma_start(ones_col[:], ones_d[:])

            x0 = bigpool.tile([128, N], F32R, tag="x0")
            nc.sync.dma_start(x0[:], x_d[0:128, :])
            x1 = bigpool.tile([128, N], F32R, tag="x1")
            nc.sync.dma_start(x1[:], x_d[128:256, :])
            xs = (x0, x1)

            th_sb = bigpool.tile([128, N], F32R, tag="th")
            ph_sb = bigpool.tile([128, N], F32R, tag="ph")
            gT_sb = bigpool.tile([128, N], F32R, tag="gT")

            # ---------------- projections: theta, phi ----------------
            for wt, bias_t, dst in ((thw, thb, th_sb), (phw, phb, ph_sb)):
                for p in range(N // QW):
                    pp = pf.tile([128, QW], F32, tag="pf")
                    for s in range(2):
                        lo = p * QW + s * 512
                        for k in range(2):
                            nc.tensor.matmul(
                                pp[:, s * 512:(s + 1) * 512],
                                wt[:, k * CI:(k + 1) * CI],
                                xs[k][:, lo:lo + 512],
                                start=(k == 0), stop=(k == 1),
                            )
                    nc.scalar.activation(
                        dst[:, p * QW:(p + 1) * QW], pp[:], AF.Identity, bias=bias_t[:]
                    )

            # ---------------- projection: gT (bias folded into wb_eff) ----------
            for j in range(JB):
                pg = pf.tile([128, 128], F32, tag="pf")
                for k in range(2):
                    nc.tensor.matmul(
                        pg[:],
                        xs[k][:, j * 128:(j + 1) * 128],
                        gw[:, k * CI:(k + 1) * CI],
                        start=(k == 0), stop=(k == 1),
                    )
                nc.scalar.activation(
                    gT_sb[:, j * 128:(j + 1) * 128], pg[:], AF.Copy
                )

            # x := x + (W_w @ g_b + W_b), per-partition scalar (after all
            # projection reads of x).
            nc.vector.tensor_scalar_add(x0[:], x0[:], wbe0[:])
            nc.vector.tensor_scalar_add(x1[:], x1[:], wbe1[:])

            # ---------------- main attention loop ----------------
            for q in range(NQ):
                i0 = q * QW
                pyt = py.tile([128, QW], F32, tag="py")
                zacc = zpool.tile([128, QW], F32R, tag="zacc")
                pzt = [None, None]
                pe_z_done = [False, False]
                n_dve = 0
                for j in range(JB):
                    pft = pf.tile([128, QW], F32, tag="pf")
                    for s in range(2):
                        nc.tensor.matmul(
                            pft[:, s * 512:(s + 1) * 512],
                            ph_sb[:, j * 128:(j + 1) * 128],
                            th_sb[:, i0 + s * 512:i0 + (s + 1) * 512],
                            start=True, stop=True,
                        )
                    ef = efpool.tile([128, QW], F32R, tag="ef")
                    nc.scalar.activation(ef[:], pft[:], AF.Exp)
                    for s in range(2):
                        nc.tensor.matmul(
                            pyt[:, s * 512:(s + 1) * 512],
                            gT_sb[:, j * 128:(j + 1) * 128],
                            ef[:, s * 512:(s + 1) * 512],
                            start=(j == 0), stop=(j == JB - 1),
                        )
                    if j in pe_z_js:
                        for s in range(2):
                            if pzt[s] is None:
                                pzt[s] = pz.tile([1, 512], F32, tag="pz",
                                                 name=f"pz_{q}_{s}")
                            nc.tensor.matmul(
                                pzt[s][:], ones_col[:],
                                ef[:, s * 512:(s + 1) * 512],
                                start=(not pe_z_done[s]), stop=False,
                            )
                            pe_z_done[s] = True
                    else:
                        if n_dve == 0:
                            nc.vector.tensor_copy(zacc[:], ef[:])
                        else:
                            nc.vector.tensor_add(zacc[:], zacc[:], ef[:])
                        n_dve += 1

                # ---- quarter tail: finish Z, normalize, project, add x ----
                zi = zpool.tile([1, QW], F32, tag="zi")
                zs = zpool.tile([1, QW], F32, tag="zs")  # recip scratch
                for s in range(2):
                    if pzt[s] is None:
                        pzt[s] = pz.tile([1, 512], F32, tag="pz",
                                         name=f"pz_{q}_{s}")
                    if n_dve:
                        nc.tensor.matmul(
                            pzt[s][:], ones_col[:],
                            zacc[:, s * 512:(s + 1) * 512],
                            start=(not pe_z_done[s]), stop=True,
                        )
                    nc.vector.reciprocal_approx_accurate(
                        zi[:, s * 512:(s + 1) * 512], pzt[s][:],
                        zs[:, s * 512:(s + 1) * 512],
                    )
                zb = zpool.tile([128, QW], F32, tag="zb")
                nc.gpsimd.partition_broadcast(zb[:], zi[:])

                # fused eviction + normalization: ynt = psum_y * (1/Z)
                ynt = ypool.tile([128, QW], F32R, tag="ynt")
                nc.vector.tensor_mul(ynt[:], pyt[:], zb[:])

                for s2 in range(2):
                    pw = py.tile([128, QW], F32, tag="py")
                    for ob in range(2):
                        nc.tensor.matmul(
                            pw[:, ob * 512:(ob + 1) * 512],
                            ww[:, ob * CI:(ob + 1) * CI],
                            ynt[:, s2 * 512:(s2 + 1) * 512],
                            start=True, stop=True,
                        )
                    for ob in range(2):
                        ot = opool.tile([128, 512], F32, tag="o")
                        nc.vector.tensor_add(
                            ot[:], pw[:, ob * 512:(ob + 1) * 512],
                            xs[ob][:, i0 + s2 * 512:i0 + (s2 + 1) * 512],
                        )
                        nc.sync.dma_start(
                            out_d[ob * 128:(ob + 1) * 128,
                                  i0 + s2 * 512:i0 + (s2 + 1) * 512],
                            ot[:],
                        )

    nc.compile()
    return nc


_CACHE = {}


def _get_nc():
    if "nc" not in _CACHE:
        _CACHE["nc"] = build()
    return _CACHE["nc"]


def _in_maps(x, g_w, g_b, theta_w, theta_b, phi_w, phi_b, W_w, W_b):
    x = np.ascontiguousarray(np.asarray(x, dtype=np.float32))
    common = {
        "thw_t": np.ascontiguousarray(np.asarray(theta_w, np.float32).T),
        "phw_t": np.ascontiguousarray(np.asarray(phi_w, np.float32).T),
        "gw_t": np.ascontiguousarray(np.asarray(g_w, np.float32).T),
        "ww_t": np.ascontiguousarray(np.asarray(W_w, np.float32).T),
        "thb": np.asarray(theta_b, np.float32).reshape(CI, 1),
        "phb": np.asarray(phi_b, np.float32).reshape(CI, 1),
        "wb_eff": (np.asarray(W_w, np.float32) @ np.asarray(g_b, np.float32)
                   + np.asarray(W_b, np.float32)).reshape(C, 1),
        "ones": np.ones((128, 1), np.float32),
    }
    return [
        {"x": np.ascontiguousarray(x[b].reshape(C, N)), **common}
        for b in range(B)
    ]


def run(in_maps, **kw):
    nc = _get_nc()
    return run_bass_kernel_spmd(nc, in_maps, list(range(B)), **kw)


def kernel(**inputs):
    res = run(_in_maps(**inputs))
    out = np.stack([res.results[b]["out"] for b in range(B)])
    return out.reshape(B, C, H, Wd)
